# revision 17
# baseline (speedup 1.0000x reference)
"""3x3 grayscale dilation (all-ones SE) = 3x3 max-pool, stride 1, zero padding.

Input (8, 3, 1024, 1024) f32 -> same-shape output.
Sharding: 24 (B*C) images, 3 per NeuronCore across 8 cores.

bf16 end-to-end: the host converts f32->bf16 before upload and bf16->f32
after download (max is monotone, so the only error is the input rounding,
~2^-9 relative -- far inside the 2e-2 gate). This halves DMA traffic AND
unlocks the DVE 2x_1p fast mode (0.5 cyc/elem) for every max op.

Per-image layout: one [128, 8192] SBUF tile; partition p holds image rows
8p..8p+7 contiguously. Measured: ~72 us vs the 112 us f32 baseline; DVE
busy ~48 us runs gap-free, the rest is fixed preamble (~6), first-load
latency (~6), store drain (~5) and the framework's semaphore-reset
teardown (~8).

Design facts (cost model + measured):
  - DVE TensorTensor gets 2x_1p (0.5 cyc/elem) only when every operand is
    2-byte dtype with a contiguous (stride +-1, >=2 elem) innermost AP dim.
    The even/odd pair trick has stride-2 operands -> stays at 1 cyc/elem, so
    in bf16 the cheapest HORIZONTAL 3-max is two contiguous-shift ops
    (t[x]=max(X[x],X[x+1]); hm[x]=max(t[x-1],t[x])): 2 ops/px at 0.5 =
    1.0 cyc/px, beating the 1.5 of the stride-2 pair trick.
  - t lives in a row-pitch-1025 buffer whose columns 0 and 1024 are zeroed
    once: the W-border max-with-pad then falls out of the same full-width
    hm instruction. No border fixup ops at all.
  - VERTICAL rows are contiguous 1024-elem runs, so the pair trick is 2x
    eligible: vp[q]=max(row 2q,row 2q+1) then one op per output row ->
    0.75 cyc/px. The hm tile has 10 row slots: ACT lands the partition-halo
    rows (see below) in slots 0 and 9, so output rows 0/7 batch into the
    same two stride-2W even/odd instructions as the interior rows; the
    whole vertical stage is 3 DVE instructions.
  - Partition-boundary halo rows come from PE shift-matmuls into PSUM
    (bf16 shifted identities, loaded from DRAM on the idle scalar ring --
    gpsimd is never used; its SWDGE DMA path measured slower). The ACT
    engine (otherwise only issuing DMA) copies PSUM f32 -> SBUF bf16.
    Max work itself can ONLY run on DVE: the compiler rejects TensorTensor
    AND TensorScalarPtr on Pool/GpSimd, ACT's activation takes
    per-partition scalars only, and scalar_tensor_tensor on DVE has no 2x
    mode (it is slower than two plain TTs).
  - DMA: two rings loading concurrently just split the ~360 GB/s fabric,
    so ALL loads ride the sync ring (up at ~6 us, ~365 GB/s solo)
    sequenced in need order -- image 0 in four row-pair quarters (first
    data ~2 us sooner), then images 1 and 2 whole (16 KiB/partition
    descriptors). All emitted before any compute so the scalar engine's
    ACT copies (which stall on PE) cannot delay a DMA issue. Stores ride
    the scalar ring; the last image drains in three row-blocks (4-7, 2-3,
    0-1) with stores split across both rings so only a 2x0.25 MiB store
    pair sits past the last DVE op.
"""

import sys

sys.path.insert(0, "/opt/trn_rl_repo")

import numpy as np
import ml_dtypes

BF16 = ml_dtypes.bfloat16

N_CORES = 8
IMGS_PER_CORE = 3
H = W = 1024
R = 8  # rows per partition
P = 128
TW = W + 1  # t-buffer row pitch (zero pad cols at 0 and W)

_COMPILED_NC = None


def _build_nc():
    import concourse.mybir as mybir
    import concourse.tile as tile
    from concourse import bacc

    bf = mybir.dt.bfloat16
    f32 = mybir.dt.float32
    MAX = mybir.AluOpType.max

    nc = bacc.Bacc(None)
    x = nc.declare_dram_parameter("input", [IMGS_PER_CORE, H, W], bf, isOutput=False)
    sdn_p = nc.declare_dram_parameter("sdn", [P, P], bf, isOutput=False)
    sup_p = nc.declare_dram_parameter("sup", [P, P], bf, isOutput=False)
    y = nc.declare_dram_parameter("output", [IMGS_PER_CORE, H, W], bf, isOutput=True)

    with tile.TileContext(nc) as tc:
        with (
            tc.tile_pool(name="xp", bufs=3) as xp,
            tc.tile_pool(name="vmp", bufs=2) as vmp,
            tc.tile_pool(name="tp", bufs=1) as tp,
            tc.tile_pool(name="hmp", bufs=2) as hmp,
            tc.tile_pool(name="vpp", bufs=2) as vpp,
            tc.tile_pool(name="shp", bufs=1) as shp,
            tc.tile_pool(name="psum", bufs=2, space="PSUM") as psp,
        ):
            # Shifted identities (sdn[k,m]=1 iff k==m-1; sup[k,m]=1 iff
            # k==m+1) come from DRAM on the scalar ring, which is otherwise
            # idle until the first store ~28 us in; the whole kernel then
            # never touches gpsimd.
            sdn = shp.tile([P, P], bf, tag="sdn")
            sup = shp.tile([P, P], bf, tag="sup")
            nc.scalar.dma_start(out=sdn[:], in_=sdn_p[:, :])
            nc.scalar.dma_start(out=sup[:], in_=sup_p[:, :])

            # Two persistent t buffers; border columns (0 and W of each row
            # block) zeroed once -- they ARE the horizontal zero padding.
            tbufs = []
            for k in range(2):
                t_ = tp.tile([P, R * TW], bf, tag=f"t{k}")
                t3 = t_[:].rearrange("p (r c) -> p r c", c=TW)
                nc.vector.memset(t3[:, :, 0:TW:W], 0.0)
                tbufs.append(t3)

            # --- all loads up front, sequenced on the sync ring in need
            # order: two rings loading concurrently just split the ~360 GB/s
            # fabric, so one ring with strict priority feeds DVE soonest ---
            Xs = []
            for i in range(IMGS_PER_CORE):
                xi = x[i].rearrange("(p r) w -> p (r w)", r=R)  # [128, 8192]
                X = xp.tile([P, R * W], bf, tag="x")
                if i == 0:
                    for q in range(0, 4):
                        nc.sync.dma_start(
                            out=X[:, 2 * q * W : 2 * (q + 1) * W],
                            in_=xi[:, 2 * q * W : 2 * (q + 1) * W],
                        )
                else:
                    nc.sync.dma_start(out=X[:], in_=xi[:, :])
                Xs.append(X)

            for i in range(IMGS_PER_CORE):
                yi = y[i].rearrange("(p r) w -> p (r w)", r=R)
                X3 = Xs[i][:].rearrange("p (r w) -> p r w", w=W)
                t3 = tbufs[i % 2]
                # hmx slot s holds: s=0 the down-halo (prev partition's hm
                # row 7), s=1..8 hm rows 0..7, s=9 the up-halo. Rows 0/7 of
                # the output then batch into the same stride-2W vertical ops.
                hmx = hmp.tile([P, (R + 2) * W], bf, tag="hm")
                hx3 = hmx[:].rearrange("p (r w) -> p r w", w=W)

                # --- horizontal 3-max: two contiguous-shift ops ---
                chunks = [(0, 2), (2, 4), (4, 6), (6, 8)] if i == 0 else [(0, R)]
                for r0, r1 in chunks:
                    nc.vector.tensor_tensor(
                        out=t3[:, r0:r1, 1:W],
                        in0=X3[:, r0:r1, 0 : W - 1],
                        in1=X3[:, r0:r1, 1:W],
                        op=MAX,
                    )
                    nc.vector.tensor_tensor(
                        out=hx3[:, r0 + 1 : r1 + 1],
                        in0=t3[:, r0:r1, 0:W],
                        in1=t3[:, r0:r1, 1 : W + 1],
                        op=MAX,
                    )

                # --- partition-boundary halo rows via PE shift-matmul ---
                # uh[p] = hm[p+1, row 0] (feeds output row 7); dh[p] =
                # hm[p-1, row 7] (feeds output row 0). Out-of-range
                # partitions give zero = the vertical padding.
                uh = psp.tile([P, W], f32, tag="uh")
                dh = psp.tile([P, W], f32, tag="dh")
                row0, row7 = W, 8 * W
                for c0 in (0, 512):
                    nc.tensor.matmul(
                        uh[:, c0 : c0 + 512],
                        sup[:],
                        hmx[:, row0 + c0 : row0 + c0 + 512],
                        start=True,
                        stop=True,
                    )
                for c0 in (0, 512):
                    nc.tensor.matmul(
                        dh[:, c0 : c0 + 512],
                        sdn[:],
                        hmx[:, row7 + c0 : row7 + c0 + 512],
                        start=True,
                        stop=True,
                    )
                # ACT lands the halos in bf16 into hmx slots 0 and 9.
                nc.scalar.copy(out=hx3[:, 9:10], in_=uh[:].rearrange("p (q w) -> p q w", w=W))
                nc.scalar.copy(out=hx3[:, 0:1], in_=dh[:].rearrange("p (q w) -> p q w", w=W))

                # --- vertical 3-max: pair trick over contiguous rows ---
                # vp[q] = max(hm 2q, hm 2q+1) = max(hmx 2q+1, hmx 2q+2)
                vp = vpp.tile([P, 4 * W], bf, tag="vp")
                vp3 = vp[:].rearrange("p (q w) -> p q w", w=W)
                vm = vmp.tile([P, R * W], bf, tag="vm")
                vm3 = vm[:].rearrange("p (r w) -> p r w", w=W)

                if i < IMGS_PER_CORE - 1:
                    nc.vector.tensor_tensor(
                        out=vp3[:, 0:4], in0=hx3[:, 1:8:2], in1=hx3[:, 2:9:2], op=MAX
                    )
                    # even rows 0,2,4,6: max(hmx[0,2,4,6], vp[0..3])
                    nc.vector.tensor_tensor(
                        out=vm3[:, 0:7:2], in0=hx3[:, 0:7:2], in1=vp3[:, 0:4], op=MAX
                    )
                    # odd rows 1,3,5,7: max(vp[0..3], hmx[3,5,7,9])
                    nc.vector.tensor_tensor(
                        out=vm3[:, 1:8:2], in0=vp3[:, 0:4], in1=hx3[:, 3:10:2], op=MAX
                    )
                    nc.scalar.dma_start(out=yi[:, :], in_=vm[:])
                else:
                    # Last image, drained in three blocks (rows 4-7, then
                    # 2-3, then 0-1) with stores split across both rings so
                    # only a 2x0.25 MiB store pair sits past the last DVE op.
                    nc.vector.tensor_tensor(
                        out=vp3[:, 2:4], in0=hx3[:, 5:8:2], in1=hx3[:, 6:9:2], op=MAX
                    )
                    nc.vector.tensor_tensor(
                        out=vm3[:, 4:7:2], in0=hx3[:, 4:7:2], in1=vp3[:, 2:4], op=MAX
                    )
                    nc.vector.tensor_tensor(
                        out=vm3[:, 5:8:2], in0=vp3[:, 2:4], in1=hx3[:, 7:10:2], op=MAX
                    )
                    nc.sync.dma_start(
                        out=yi[:, 4 * W : 6 * W], in_=vm[:, 4 * W : 6 * W]
                    )
                    nc.scalar.dma_start(
                        out=yi[:, 6 * W : 8 * W], in_=vm[:, 6 * W : 8 * W]
                    )
                    # rows 2-3: vp pair (2,3) in vp3 slot 1
                    nc.vector.tensor_tensor(
                        out=vp3[:, 1:2], in0=hx3[:, 3:4], in1=hx3[:, 4:5], op=MAX
                    )
                    nc.vector.tensor_tensor(
                        out=vm3[:, 2:3], in0=hx3[:, 2:3], in1=vp3[:, 1:2], op=MAX
                    )
                    nc.vector.tensor_tensor(
                        out=vm3[:, 3:4], in0=vp3[:, 1:2], in1=hx3[:, 5:6], op=MAX
                    )
                    nc.sync.dma_start(
                        out=yi[:, 2 * W : 3 * W], in_=vm[:, 2 * W : 3 * W]
                    )
                    nc.scalar.dma_start(
                        out=yi[:, 3 * W : 4 * W], in_=vm[:, 3 * W : 4 * W]
                    )
                    # rows 0-1: vp pair (0,1) in vp3 slot 0
                    nc.vector.tensor_tensor(
                        out=vp3[:, 0:1], in0=hx3[:, 1:2], in1=hx3[:, 2:3], op=MAX
                    )
                    nc.vector.tensor_tensor(
                        out=vm3[:, 1:2], in0=vp3[:, 0:1], in1=hx3[:, 3:4], op=MAX
                    )
                    nc.vector.tensor_tensor(
                        out=vm3[:, 0:1], in0=hx3[:, 0:1], in1=vp3[:, 0:1], op=MAX
                    )
                    nc.sync.dma_start(out=yi[:, 0:W], in_=vm[:, 0:W])
                    nc.scalar.dma_start(out=yi[:, W : 2 * W], in_=vm[:, W : 2 * W])

    nc.compile()
    return nc


def _get_nc():
    global _COMPILED_NC
    if _COMPILED_NC is None:
        _COMPILED_NC = _build_nc()
    return _COMPILED_NC


def _to_bf16_in_maps(input_f32):
    flat = np.ascontiguousarray(input_f32.reshape(N_CORES * IMGS_PER_CORE, H, W))
    flat = flat.astype(BF16)
    # sdn[k, m] = 1 iff k == m-1 ; sup[k, m] = 1 iff k == m+1
    sdn = np.eye(P, k=1, dtype=np.float32).astype(BF16)
    sup = np.eye(P, k=-1, dtype=np.float32).astype(BF16)
    return [
        {
            "input": flat[k * IMGS_PER_CORE : (k + 1) * IMGS_PER_CORE],
            "sdn": sdn,
            "sup": sup,
        }
        for k in range(N_CORES)
    ]


def _reference_fallback(input, se):
    # Generic path (never hit for the graded all-ones 3x3 se); mirrors the
    # kornia Dilate reference exactly.
    se = np.asarray(se, dtype=np.float32)
    se_h, se_w = se.shape
    pad_h, pad_w = se_h // 2, se_w // 2
    B, C, Hh, Ww = input.shape
    se_m1 = (se - 1.0).reshape(-1)
    padded = np.pad(input, ((0, 0), (0, 0), (pad_h, pad_h), (pad_w, pad_w)))
    out = None
    for i in range(se_h * se_w):
        xs, ys = i // se_h, i % se_h
        mask = np.float32(1.0) if se_m1[i] >= 0 else np.float32(0.0)
        contrib = mask * padded[:, :, xs : xs + Hh, ys : ys + Ww] + se_m1[i]
        out = contrib if out is None else np.maximum(out, contrib)
    return out


def kernel(input, se):
    from concourse.bass_utils import run_bass_kernel_spmd

    input = np.ascontiguousarray(np.asarray(input, dtype=np.float32))
    se_np = np.asarray(se, dtype=np.float32)
    if se_np.shape != (3, 3) or not np.all(se_np == 1.0) or input.shape != (
        8,
        3,
        H,
        W,
    ):
        return _reference_fallback(input, se_np).astype(np.float32)

    nc = _get_nc()
    in_maps = _to_bf16_in_maps(input)
    last_err = None
    for _attempt in range(3):
        try:
            res = run_bass_kernel_spmd(nc, in_maps, list(range(N_CORES)))
            out = np.concatenate(
                [np.asarray(res.results[k]["output"]) for k in range(N_CORES)],
                axis=0,
            )
            return out.astype(np.float32).reshape(8, 3, H, W)
        except Exception as e:  # transient NRT_EXEC_UNIT_UNRECOVERABLE etc.
            last_err = e
    raise last_err


# revision 19
# speedup vs baseline: 1.0323x; 1.0323x over previous
"""3x3 grayscale dilation (all-ones SE) = 3x3 max-pool, stride 1, zero padding.

Input (8, 3, 1024, 1024) f32 -> same-shape output.
Sharding: 24 (B*C) images, 3 per NeuronCore across 8 cores.

bf16 end-to-end: the host converts f32->bf16 before upload and bf16->f32
after download (max is monotone, so the only error is the input rounding,
~2^-9 relative -- far inside the 2e-2 gate). This halves DMA traffic AND
unlocks the DVE 2x_1p fast mode (0.5 cyc/elem) for every max op.

Per-image layout: one [128, 8192] SBUF tile; partition p holds image rows
8p..8p+7 contiguously. Measured: ~72 us vs the 112 us f32 baseline; DVE
busy ~48 us runs gap-free, the rest is fixed preamble (~6), first-load
latency (~6), store drain (~5) and the framework's semaphore-reset
teardown (~8).

Design facts (cost model + measured):
  - DVE TensorTensor gets 2x_1p (0.5 cyc/elem) only when every operand is
    2-byte dtype with a contiguous (stride +-1, >=2 elem) innermost AP dim.
    The even/odd pair trick has stride-2 operands -> stays at 1 cyc/elem, so
    in bf16 the cheapest HORIZONTAL 3-max is two contiguous-shift ops
    (t[x]=max(X[x],X[x+1]); hm[x]=max(t[x-1],t[x])): 2 ops/px at 0.5 =
    1.0 cyc/px, beating the 1.5 of the stride-2 pair trick.
  - t lives in a row-pitch-1025 buffer whose columns 0 and 1024 are zeroed
    once: the W-border max-with-pad then falls out of the same full-width
    hm instruction. No border fixup ops at all.
  - VERTICAL rows are contiguous 1024-elem runs, so the pair trick is 2x
    eligible: vp[q]=max(row 2q,row 2q+1) then one op per output row ->
    0.75 cyc/px. The hm tile has 10 row slots: ACT lands the partition-halo
    rows (see below) in slots 0 and 9, so output rows 0/7 batch into the
    same two stride-2W even/odd instructions as the interior rows; the
    whole vertical stage is 3 DVE instructions.
  - Partition-boundary halo rows come from PE shift-matmuls into PSUM
    (bf16 shifted identities, loaded from DRAM on the idle scalar ring --
    gpsimd is never used; its SWDGE DMA path measured slower). The ACT
    engine (otherwise only issuing DMA) copies PSUM f32 -> SBUF bf16.
    Max work itself can ONLY run on DVE: the compiler rejects TensorTensor
    AND TensorScalarPtr on Pool/GpSimd, ACT's activation takes
    per-partition scalars only, and scalar_tensor_tensor on DVE has no 2x
    mode (it is slower than two plain TTs).
  - DMA: two rings loading concurrently just split the ~360 GB/s fabric,
    so ALL loads ride the sync ring (up at ~6 us, ~365 GB/s solo)
    sequenced in need order -- image 0 in four row-pair quarters (first
    data ~2 us sooner), then images 1 and 2 whole (16 KiB/partition
    descriptors). All emitted before any compute so the scalar engine's
    ACT copies (which stall on PE) cannot delay a DMA issue. Stores ride
    the scalar ring; the last image drains in three row-blocks (4-7, 2-3,
    0-1) with stores split across both rings so only a 2x0.25 MiB store
    pair sits past the last DVE op.
"""

import sys

sys.path.insert(0, "/opt/trn_rl_repo")

import numpy as np
import ml_dtypes

BF16 = ml_dtypes.bfloat16

N_CORES = 8
IMGS_PER_CORE = 3
H = W = 1024
R = 8  # rows per partition
P = 128
TW = W + 1  # t-buffer row pitch (zero pad cols at 0 and W)

_COMPILED_NC = None


def _build_nc():
    import concourse.mybir as mybir
    import concourse.tile as tile
    from concourse import bacc

    bf = mybir.dt.bfloat16
    f32 = mybir.dt.float32
    MAX = mybir.AluOpType.max

    nc = bacc.Bacc(None)
    x = nc.declare_dram_parameter("input", [IMGS_PER_CORE, H, W], bf, isOutput=False)
    sdn_p = nc.declare_dram_parameter("sdn", [P, P], bf, isOutput=False)
    sup_p = nc.declare_dram_parameter("sup", [P, P], bf, isOutput=False)
    y = nc.declare_dram_parameter("output", [IMGS_PER_CORE, H, W], bf, isOutput=True)

    with tile.TileContext(nc) as tc:
        with (
            tc.tile_pool(name="xp", bufs=3) as xp,
            tc.tile_pool(name="vmp", bufs=2) as vmp,
            tc.tile_pool(name="tp", bufs=1) as tp,
            tc.tile_pool(name="hmp", bufs=2) as hmp,
            tc.tile_pool(name="vpp", bufs=2) as vpp,
            tc.tile_pool(name="shp", bufs=1) as shp,
            tc.tile_pool(name="psum", bufs=2, space="PSUM") as psp,
        ):
            # Shifted identities (sdn[k,m]=1 iff k==m-1; sup[k,m]=1 iff
            # k==m+1) come from DRAM on the scalar ring, which is otherwise
            # idle until the first store ~28 us in; the whole kernel then
            # never touches gpsimd.
            sdn = shp.tile([P, P], bf, tag="sdn")
            sup = shp.tile([P, P], bf, tag="sup")
            nc.scalar.dma_start(out=sdn[:], in_=sdn_p[:, :])
            nc.scalar.dma_start(out=sup[:], in_=sup_p[:, :])

            # Two persistent t buffers; border columns (0 and W of each row
            # block) zeroed once -- they ARE the horizontal zero padding.
            tbufs = []
            for k in range(2):
                t_ = tp.tile([P, R * TW], bf, tag=f"t{k}")
                t3 = t_[:].rearrange("p (r c) -> p r c", c=TW)
                nc.vector.memset(t3[:, :, 0:TW:W], 0.0)
                tbufs.append(t3)

            # --- all loads up front, sequenced on the sync ring in need
            # order: two rings loading concurrently just split the ~360 GB/s
            # fabric, so one ring with strict priority feeds DVE soonest ---
            Xs = []
            for i in range(IMGS_PER_CORE):
                xi = x[i].rearrange("(p r) w -> p (r w)", r=R)  # [128, 8192]
                X = xp.tile([P, R * W], bf, tag="x")
                if i == 0:
                    # Small first chunk for the earliest DVE start; the rest
                    # in one transfer that lands just as the first chunk's
                    # horizontal work drains.
                    nc.sync.dma_start(out=X[:, 0 : 2 * W], in_=xi[:, 0 : 2 * W])
                    nc.sync.dma_start(out=X[:, 2 * W :], in_=xi[:, 2 * W :])
                else:
                    nc.sync.dma_start(out=X[:], in_=xi[:, :])
                Xs.append(X)

            for i in range(IMGS_PER_CORE):
                yi = y[i].rearrange("(p r) w -> p (r w)", r=R)
                X3 = Xs[i][:].rearrange("p (r w) -> p r w", w=W)
                t3 = tbufs[i % 2]
                # hmx slot s holds: s=0 the down-halo (prev partition's hm
                # row 7), s=1..8 hm rows 0..7, s=9 the up-halo. Rows 0/7 of
                # the output then batch into the same stride-2W vertical ops.
                hmx = hmp.tile([P, (R + 2) * W], bf, tag="hm")
                hx3 = hmx[:].rearrange("p (r w) -> p r w", w=W)

                # --- horizontal 3-max: two contiguous-shift ops ---
                chunks = [(0, 2), (2, 8)] if i == 0 else [(0, R)]
                for r0, r1 in chunks:
                    nc.vector.tensor_tensor(
                        out=t3[:, r0:r1, 1:W],
                        in0=X3[:, r0:r1, 0 : W - 1],
                        in1=X3[:, r0:r1, 1:W],
                        op=MAX,
                    )
                    nc.vector.tensor_tensor(
                        out=hx3[:, r0 + 1 : r1 + 1],
                        in0=t3[:, r0:r1, 0:W],
                        in1=t3[:, r0:r1, 1 : W + 1],
                        op=MAX,
                    )

                # --- partition-boundary halo rows via PE shift-matmul ---
                # uh[p] = hm[p+1, row 0] (feeds output row 7); dh[p] =
                # hm[p-1, row 7] (feeds output row 0). Out-of-range
                # partitions give zero = the vertical padding.
                uh = psp.tile([P, W], f32, tag="uh")
                dh = psp.tile([P, W], f32, tag="dh")
                row0, row7 = W, 8 * W
                for c0 in (0, 512):
                    nc.tensor.matmul(
                        uh[:, c0 : c0 + 512],
                        sup[:],
                        hmx[:, row0 + c0 : row0 + c0 + 512],
                        start=True,
                        stop=True,
                    )
                for c0 in (0, 512):
                    nc.tensor.matmul(
                        dh[:, c0 : c0 + 512],
                        sdn[:],
                        hmx[:, row7 + c0 : row7 + c0 + 512],
                        start=True,
                        stop=True,
                    )
                # ACT lands the halos in bf16 into hmx slots 0 and 9.
                nc.scalar.copy(out=hx3[:, 9:10], in_=uh[:].rearrange("p (q w) -> p q w", w=W))
                nc.scalar.copy(out=hx3[:, 0:1], in_=dh[:].rearrange("p (q w) -> p q w", w=W))

                # --- vertical 3-max: pair trick over contiguous rows ---
                # vp[q] = max(hm 2q, hm 2q+1) = max(hmx 2q+1, hmx 2q+2)
                vp = vpp.tile([P, 4 * W], bf, tag="vp")
                vp3 = vp[:].rearrange("p (q w) -> p q w", w=W)
                vm = vmp.tile([P, R * W], bf, tag="vm")
                vm3 = vm[:].rearrange("p (r w) -> p r w", w=W)

                if i < IMGS_PER_CORE - 1:
                    nc.vector.tensor_tensor(
                        out=vp3[:, 0:4], in0=hx3[:, 1:8:2], in1=hx3[:, 2:9:2], op=MAX
                    )
                    # even rows 0,2,4,6: max(hmx[0,2,4,6], vp[0..3])
                    nc.vector.tensor_tensor(
                        out=vm3[:, 0:7:2], in0=hx3[:, 0:7:2], in1=vp3[:, 0:4], op=MAX
                    )
                    # odd rows 1,3,5,7: max(vp[0..3], hmx[3,5,7,9])
                    nc.vector.tensor_tensor(
                        out=vm3[:, 1:8:2], in0=vp3[:, 0:4], in1=hx3[:, 3:10:2], op=MAX
                    )
                    nc.scalar.dma_start(out=yi[:, :], in_=vm[:])
                else:
                    # Last image, drained in three blocks (rows 4-7, then
                    # 2-3, then 0-1) with stores split across both rings so
                    # only a 2x0.25 MiB store pair sits past the last DVE op.
                    nc.vector.tensor_tensor(
                        out=vp3[:, 2:4], in0=hx3[:, 5:8:2], in1=hx3[:, 6:9:2], op=MAX
                    )
                    nc.vector.tensor_tensor(
                        out=vm3[:, 4:7:2], in0=hx3[:, 4:7:2], in1=vp3[:, 2:4], op=MAX
                    )
                    nc.vector.tensor_tensor(
                        out=vm3[:, 5:8:2], in0=vp3[:, 2:4], in1=hx3[:, 7:10:2], op=MAX
                    )
                    nc.sync.dma_start(
                        out=yi[:, 4 * W : 6 * W], in_=vm[:, 4 * W : 6 * W]
                    )
                    nc.scalar.dma_start(
                        out=yi[:, 6 * W : 8 * W], in_=vm[:, 6 * W : 8 * W]
                    )
                    # rows 2-3: vp pair (2,3) in vp3 slot 1
                    nc.vector.tensor_tensor(
                        out=vp3[:, 1:2], in0=hx3[:, 3:4], in1=hx3[:, 4:5], op=MAX
                    )
                    nc.vector.tensor_tensor(
                        out=vm3[:, 2:3], in0=hx3[:, 2:3], in1=vp3[:, 1:2], op=MAX
                    )
                    nc.vector.tensor_tensor(
                        out=vm3[:, 3:4], in0=vp3[:, 1:2], in1=hx3[:, 5:6], op=MAX
                    )
                    nc.sync.dma_start(
                        out=yi[:, 2 * W : 3 * W], in_=vm[:, 2 * W : 3 * W]
                    )
                    nc.scalar.dma_start(
                        out=yi[:, 3 * W : 4 * W], in_=vm[:, 3 * W : 4 * W]
                    )
                    # rows 0-1: vp pair (0,1) in vp3 slot 0
                    nc.vector.tensor_tensor(
                        out=vp3[:, 0:1], in0=hx3[:, 1:2], in1=hx3[:, 2:3], op=MAX
                    )
                    nc.vector.tensor_tensor(
                        out=vm3[:, 1:2], in0=vp3[:, 0:1], in1=hx3[:, 3:4], op=MAX
                    )
                    nc.vector.tensor_tensor(
                        out=vm3[:, 0:1], in0=hx3[:, 0:1], in1=vp3[:, 0:1], op=MAX
                    )
                    nc.sync.dma_start(out=yi[:, 0:W], in_=vm[:, 0:W])
                    nc.scalar.dma_start(out=yi[:, W : 2 * W], in_=vm[:, W : 2 * W])

    nc.compile()
    return nc


def _get_nc():
    global _COMPILED_NC
    if _COMPILED_NC is None:
        _COMPILED_NC = _build_nc()
    return _COMPILED_NC


def _to_bf16_in_maps(input_f32):
    flat = np.ascontiguousarray(input_f32.reshape(N_CORES * IMGS_PER_CORE, H, W))
    flat = flat.astype(BF16)
    # sdn[k, m] = 1 iff k == m-1 ; sup[k, m] = 1 iff k == m+1
    sdn = np.eye(P, k=1, dtype=np.float32).astype(BF16)
    sup = np.eye(P, k=-1, dtype=np.float32).astype(BF16)
    return [
        {
            "input": flat[k * IMGS_PER_CORE : (k + 1) * IMGS_PER_CORE],
            "sdn": sdn,
            "sup": sup,
        }
        for k in range(N_CORES)
    ]


def _reference_fallback(input, se):
    # Generic path (never hit for the graded all-ones 3x3 se); mirrors the
    # kornia Dilate reference exactly.
    se = np.asarray(se, dtype=np.float32)
    se_h, se_w = se.shape
    pad_h, pad_w = se_h // 2, se_w // 2
    B, C, Hh, Ww = input.shape
    se_m1 = (se - 1.0).reshape(-1)
    padded = np.pad(input, ((0, 0), (0, 0), (pad_h, pad_h), (pad_w, pad_w)))
    out = None
    for i in range(se_h * se_w):
        xs, ys = i // se_h, i % se_h
        mask = np.float32(1.0) if se_m1[i] >= 0 else np.float32(0.0)
        contrib = mask * padded[:, :, xs : xs + Hh, ys : ys + Ww] + se_m1[i]
        out = contrib if out is None else np.maximum(out, contrib)
    return out


def kernel(input, se):
    from concourse.bass_utils import run_bass_kernel_spmd

    input = np.ascontiguousarray(np.asarray(input, dtype=np.float32))
    se_np = np.asarray(se, dtype=np.float32)
    if se_np.shape != (3, 3) or not np.all(se_np == 1.0) or input.shape != (
        8,
        3,
        H,
        W,
    ):
        return _reference_fallback(input, se_np).astype(np.float32)

    nc = _get_nc()
    in_maps = _to_bf16_in_maps(input)
    last_err = None
    for _attempt in range(3):
        try:
            res = run_bass_kernel_spmd(nc, in_maps, list(range(N_CORES)))
            out = np.concatenate(
                [np.asarray(res.results[k]["output"]) for k in range(N_CORES)],
                axis=0,
            )
            return out.astype(np.float32).reshape(8, 3, H, W)
        except Exception as e:  # transient NRT_EXEC_UNIT_UNRECOVERABLE etc.
            last_err = e
    raise last_err
